# revision 31
# baseline (speedup 1.0000x reference)
"""AttentionalPropagation (SuperGlue-style GNN message passing) on 8 TRN2 NeuronCores.

Sharding: pure data parallel over the batch dim (B=8 -> one batch element per core).
Per-core computation (x, src are (256, 2048) slices; all matmuls in bf16, f32 accum):

  Q = WqS @ x + bq          (256, 2048)   stacked-head layout, c = h*64+dh
  K = WkS @ s + bk          (256, 2048)
  VT = s^T @ WvS^T + bv     (2048, 256)   keys on partitions (transposed layout)
  per head h: S^T[m,n] = K_h[:,m] . Q_h[:,n]  -> exp(S^T/8)  (no max-subtraction;
      scores are O(1) so exp is safe)
  msg_u[dh,n] = sum_m exp . VT[m, h*64+dh]  (col-packed head pairs)
  den[n] = sum_m exp                        (4-way col-packed ones-matmuls)
  msg = msg_u / den
  h1 = W1x @ x + (W1m@WmP) @ msg   (Wm folded into W1 on host; b1/bm-terms cancel
                                    in InstanceNorm)
  hn = relu(h1 - mean);  out = (W2 * rstd) @ hn + b2   (rstd>0 commutes with relu)

Scheduling: software-pipelined one n-chunk back AND interleaved at super-tile
granularity (scores for chunk j alternate with msg/den for chunk j-1 in the PE
stream), m-accumulation chains run reversed so Tile emits at most one semaphore
wait per chain.
"""

import os
import sys

for _p in ("/opt/trn_rl_repo",):
    if _p not in sys.path:
        sys.path.insert(0, _p)

import numpy as np
import ml_dtypes

import concourse.bass as bass
import concourse.mybir as mybir
from concourse import bacc
from concourse import library_config
from concourse.bass import ts
from concourse.tile import TileContext
from concourse.bass_utils import run_bass_kernel_spmd

F32 = mybir.dt.float32
BF16 = mybir.dt.bfloat16
AF = mybir.ActivationFunctionType
ALU = mybir.AluOpType

B, D, N, M, H, DH = 8, 256, 2048, 2048, 4, 64
EPS = 1e-5
NCH = 4  # n-chunks of 512
CHUNK = 512


def _build():
    nc = bacc.Bacc("TRN2", target_bir_lowering=False, debug=False, num_devices=8)

    x_d = nc.dram_tensor("x", [2, 128, N], BF16, kind="ExternalInput").ap()
    s_d = nc.dram_tensor("src", [2, 128, M], BF16, kind="ExternalInput").ap()
    wq_d = nc.dram_tensor("wqT", [2, 128, D], BF16, kind="ExternalInput").ap()
    wk_d = nc.dram_tensor("wkT", [2, 128, D], BF16, kind="ExternalInput").ap()
    wv_d = nc.dram_tensor("wvT", [2, 128, D], BF16, kind="ExternalInput").ap()
    w1_d = nc.dram_tensor("w1T", [4, 128, 2 * D], BF16, kind="ExternalInput").ap()
    w2_d = nc.dram_tensor("w2T", [4, 128, D], BF16, kind="ExternalInput").ap()
    # biases packed as columns: [bq, bk, b2]
    bias_d = nc.dram_tensor("bias", [2, 128, 3], F32, kind="ExternalInput").ap()
    bv_d = nc.dram_tensor("bv", [1, D], BF16, kind="ExternalInput").ap()
    out_d = nc.dram_tensor("out", [D, N], F32, kind="ExternalOutput").ap()

    with TileContext(nc) as tc:
        nc.gpsimd.load_library(library_config.attn)
        with (
            tc.tile_pool(name="const", bufs=1) as const,
            tc.tile_pool(name="data", bufs=1) as data,
            tc.tile_pool(name="reuse", bufs=2) as reuse,
            tc.tile_pool(name="exps", bufs=6) as exps,
            tc.tile_pool(name="small", bufs=2) as small,
            tc.tile_pool(name="msgn", bufs=4) as msgn,
            tc.tile_pool(name="ps_sc", bufs=3, space="PSUM") as ps_sc,
            tc.tile_pool(name="ps_shared", bufs=2, space="PSUM") as ps_shared,
        ):
            # ---- inputs + weights (few large DMAs; x/wq first for fast start) ----
            x_sb = data.tile([128, 2, N], BF16, name="x")
            wq_sb = const.tile([128, 2, D], BF16, name="wq")
            nc.sync.dma_start(out=x_sb[:], in_=x_d.rearrange("k p n -> p k n"))
            nc.sync.dma_start(out=wq_sb[:], in_=wq_d.rearrange("k p n -> p k n"))
            s_sb = reuse.tile([128, 2, M], BF16, name="s", tag="big")
            wk_sb = const.tile([128, 2, D], BF16, name="wk")
            wv_sb = const.tile([128, 2, D], BF16, name="wv")
            nc.sync.dma_start(out=s_sb[:], in_=s_d.rearrange("k p n -> p k n"))
            nc.sync.dma_start(out=wk_sb[:], in_=wk_d.rearrange("k p n -> p k n"))
            nc.sync.dma_start(out=wv_sb[:], in_=wv_d.rearrange("k p n -> p k n"))
            bias_sb = const.tile([128, 2, 3], F32, name="bias")
            nc.sync.dma_start(out=bias_sb[:], in_=bias_d.rearrange("k p n -> p k n"))
            bv_bc = const.tile([128, D], BF16, name="bvbc")
            bv_src = bass.AP(
                tensor=bv_d.tensor, offset=bv_d.offset, ap=[[0, 128]] + bv_d.ap[1:]
            )
            nc.sync.dma_start(out=bv_bc[:], in_=bv_src)
            w1_sb = const.tile([128, 4, 2 * D], BF16, name="w1")
            nc.sync.dma_start(out=w1_sb[:], in_=w1_d.rearrange("k p n -> p k n"))
            w2_sb = const.tile([128, 4, D], BF16, name="w2")
            nc.sync.dma_start(out=w2_sb[:], in_=w2_d.rearrange("k p n -> p k n"))
            eps_sb = const.tile([128, 1], F32, name="eps")
            nc.vector.memset(eps_sb[:], EPS)
            ones_sb = const.tile([128, 1], BF16, name="ones")
            nc.vector.memset(ones_sb[:], 1.0)

            # PE warmup: dummy matmuls on uninitialized SBUF while DMAs land,
            # so the HAM clock gate opens before the real QKV matmuls start
            dummy_sb = const.tile([128, 128], BF16, name="dummy")
            nc.vector.memset(dummy_sb[:], 0.0)
            wup = ps_shared.tile([128, 128], F32, name="wup", tag="sps")
            for _ in range(12):
                nc.tensor.matmul(wup[:], dummy_sb[:], dummy_sb[:],
                                 start=True, stop=True)

            # ---- QKV projections (weight-stationary: 1 LDW per 4 MMs) ----
            q_sb = data.tile([128, 2, N], BF16, name="q")
            k_sb = data.tile([128, 2, M], BF16, name="k")

            def emit_qk(c, skip_j0=False):
                for dst, w_sb, src_t, b_col in (
                    (q_sb, wq_sb, x_sb, 0),
                    (k_sb, wk_sb, s_sb, 1),
                ):
                    ps = [
                        ps_sc.tile([128, 2, CHUNK], F32, name="qk", tag="scps")
                        for _ in range(2)
                    ]
                    for k in range(2):
                        for j in range(NCH):
                            if skip_j0 and j == 0:
                                continue
                            nc.tensor.matmul(
                                ps[j // 2][:, j % 2, :],
                                w_sb[:, k, ts(c, 128)],
                                src_t[:, k, ts(j, CHUNK)],
                                start=(k == 0),
                                stop=(k == 1),
                            )
                    nc.vector.tensor_scalar_add(
                        dst[:, c, CHUNK if skip_j0 else 0 : 2 * CHUNK],
                        ps[0][:, 1:2, :] if skip_j0 else ps[0][:],
                        bias_sb[:, c, b_col : b_col + 1],
                    )
                    nc.vector.tensor_scalar_add(
                        dst[:, c, ts(1, 2 * CHUNK)],
                        ps[1][:],
                        bias_sb[:, c, b_col : b_col + 1],
                    )

            def emit_qk0_rest(c):
                emit_qk(c, skip_j0=True)

            # V^T: (m, c) layout, 65-wide per-head blocks with a ones column
            vT_sb = [data.tile([128, H, DH + 1], BF16, name=f"vT{t}")
                     for t in range(16)]

            def emit_vT(trange):
                for t in trange:
                    vp = ps_shared.tile([128, D], F32, name="vps", tag="sps")
                    for k in range(2):
                        nc.tensor.matmul(
                            vp[:],
                            s_sb[:, k, ts(t, 128)],
                            wv_sb[:, k, :],
                            start=(k == 0),
                            stop=(k == 1),
                        )
                    nc.vector.tensor_add(
                        vT_sb[t][:, :, 0:DH],
                        vp[:].rearrange("p (h d) -> p h d", h=H),
                        bv_bc[:].rearrange("p (h d) -> p h d", h=H),
                    )
                    nc.vector.memset(vT_sb[t][:, :, DH : DH + 1], 1.0)

            # ---- attention ----
            h1_sb = data.tile([128, 4, N], BF16, name="h1")
            stats_sb = data.tile([128, 4, NCH, 6], F32, name="stats")
            eS = {}  # (j, h, half) -> expS tile (128, 8, CHUNK)
            mn = {}  # (j, p) -> normalized msg pair tile (128, CHUNK)
            mps = {}  # (j, p) -> msg psum ; (j, 'd') -> den psum

            def emit_scores_super(j, p, s):
                # scores + exp for super-tile s (m-tiles 2s, 2s+1), head pair p
                if s == 0:
                    for h2 in range(2):
                        eS[(j, 2 * p + h2)] = exps.tile(
                            [128, 16, CHUNK], BF16, name="expS", tag="expS"
                        )
                scp = [
                    ps_sc.tile([128, 2, CHUNK], F32, name="sc", tag="scps")
                    for _ in range(2)
                ]
                for jj in range(2):
                    mt = 2 * s + jj
                    for h2 in range(2):
                        nc.tensor.matmul(
                            scp[h2][:, jj, :],
                            k_sb[ts(h2, DH), p, ts(mt, 128)],
                            q_sb[ts(h2, DH), p, ts(j, CHUNK)],
                            start=True,
                            stop=True,
                        )
                for h2 in range(2):
                    nc.scalar.activation(
                        eS[(j, 2 * p + h2)][:, 2 * s : 2 * s + 2, :],
                        scp[h2][:],
                        AF.Exp,
                        scale=1.0 / 8.0,
                    )

            def emit_msg_head(j, h):
                # augmented-V msg chain (psum row 64 = denominator)
                p, h2 = h // 2, h % 2
                if h2 == 0:
                    mn[(j, p)] = msgn.tile([128, CHUNK], BF16, name="mn", tag="mn")
                mp = ps_shared.tile([DH + 1, CHUNK], F32, name="msgps", tag="sps")
                for mt in range(16):
                    nc.tensor.matmul(
                        mp[:],
                        vT_sb[mt][:, h, :],
                        eS[(j, h)][:, mt, :],
                        start=(mt == 0),
                        stop=(mt == 15),
                    )
                del eS[(j, h)]
                den = small.tile([1, CHUNK], F32, name="den", tag="den")
                nc.vector.tensor_copy(den[:], mp[DH : DH + 1, :])
                rden = small.tile([1, CHUNK], F32, name="rden", tag="rden")
                nc.vector.reciprocal_approx_fast(rden[:], den[:])
                rbc = small.tile([DH, CHUNK], F32, name="rbc", tag="rbc")
                nc.gpsimd.partition_broadcast(rbc[:], rden[:])
                nc.vector.tensor_mul(mn[(j, p)][ts(h2, DH), :], mp[0:DH, :], rbc[:])

            def emit_norm_h1(j):
                # h1 = W1x @ x + W1mWm @ msg
                for o in range(4):
                    hp = ps_shared.tile([128, CHUNK], F32, name="h1ps", tag="sps")
                    for k in range(4):
                        rhs = (
                            x_sb[:, k, ts(j, CHUNK)] if k < 2 else mn[(j, k - 2)][:]
                        )
                        nc.tensor.matmul(
                            hp[:],
                            w1_sb[:, k, ts(o, 128)],
                            rhs,
                            start=(k == 0),
                            stop=(k == 3),
                        )
                    nc.vector.tensor_copy(h1_sb[:, o, ts(j, CHUNK)], hp[:])
                    nc.vector.bn_stats(
                        stats_sb[:, o, j, :], h1_sb[:, o, ts(j, CHUNK)]
                    )

            # ---- schedule ----
            for dst, w_sb, src_t, b_col in (
                (q_sb, wq_sb, x_sb, 0),
                (k_sb, wk_sb, s_sb, 1),
            ):
                pfx = ps_shared.tile([128, CHUNK], F32, name="pfx", tag="sps")
                for k in range(2):
                    nc.tensor.matmul(
                        pfx[:],
                        w_sb[:, k, 0:128],
                        src_t[:, k, 0:CHUNK],
                        start=(k == 0),
                        stop=(k == 1),
                    )
                nc.vector.tensor_scalar_add(
                    dst[:, 0, 0:CHUNK], pfx[:], bias_sb[:, 0, b_col : b_col + 1]
                )
            emit_scores_super(0, 0, 0)
            emit_qk0_rest(0)
            for s in range(1, 8):
                emit_scores_super(0, 0, s)
            emit_qk(1)
            for s in range(8):
                emit_scores_super(0, 1, s)
                emit_vT(range(2 * s, 2 * s + 2))
            for j in range(1, NCH - 1):
                for s in range(8):
                    emit_scores_super(j, 0, s)
                    emit_scores_super(j, 1, s)
                for h in range(4):
                    emit_msg_head(j - 1, h)
                emit_norm_h1(j - 1)
            jL = NCH - 1
            for s in range(8):
                emit_scores_super(jL, 0, s)
            for s in range(8):
                emit_scores_super(jL, 1, s)
            for h in range(4):
                emit_msg_head(jL - 1, h)
            emit_norm_h1(jL - 1)
            for h in range(4):
                emit_msg_head(jL, h)
            emit_norm_h1(jL)

            # ---- InstanceNorm (relu on DVE, rstd folded into W2) + W2 ----
            for _ in range(20):
                wup2 = ps_shared.tile([128, 128], F32, name="wup2", tag="sps")
                nc.tensor.matmul(wup2[:], dummy_sb[:], dummy_sb[:],
                                 start=True, stop=True)
            hn_sb = reuse.tile([128, 4, N], BF16, name="hn", tag="big")
            nmean = small.tile([128, 4], F32, name="nmean", tag="mean")
            var4 = small.tile([128, 4], F32, name="var4", tag="var4")
            for o in range(4):
                mv = small.tile([128, 2], F32, name="mv", tag="mv")
                nc.vector.bn_aggr(mv[:], stats_sb[:, o, :, :])
                nc.vector.tensor_scalar_mul(nmean[:, o : o + 1], mv[:, 0:1], -1.0)
                nc.vector.tensor_copy(var4[:, o : o + 1], mv[:, 1:2])
            lv4 = small.tile([128, 4], F32, name="lv4", tag="std4")
            nc.scalar.activation(lv4[:], var4[:], AF.Ln, bias=eps_sb[:])
            rstd4 = small.tile([128, 4], F32, name="rstd4", tag="rstd4")
            nc.scalar.activation(rstd4[:], lv4[:], AF.Exp, scale=-0.5)
            for o in range(4):
                nc.vector.tensor_scalar_mul(
                    w2_sb[:, o, :], w2_sb[:, o, :], rstd4[:, o : o + 1]
                )
            for j in range(NCH):
                for o in range(4):
                    if o % 2 == 0:
                        nc.scalar.activation(
                            hn_sb[:, o, ts(j, CHUNK)],
                            h1_sb[:, o, ts(j, CHUNK)],
                            AF.Relu,
                            bias=nmean[:, o : o + 1],
                        )
                    else:
                        nc.vector.tensor_scalar(
                            hn_sb[:, o, ts(j, CHUNK)],
                            h1_sb[:, o, ts(j, CHUNK)],
                            nmean[:, o : o + 1],
                            0.0,
                            op0=ALU.add,
                            op1=ALU.max,
                        )
                for c in range(2):
                    op = ps_shared.tile([128, CHUNK], F32, name="ops", tag="sps")
                    for ki, k in enumerate((3, 2, 1, 0)):
                        nc.tensor.matmul(
                            op[:],
                            w2_sb[:, k, ts(c, 128)],
                            hn_sb[:, k, ts(j, CHUNK)],
                            start=(ki == 0),
                            stop=(ki == 3),
                        )
                    ot = small.tile([128, CHUNK], F32, name="outt", tag="outt")
                    nc.scalar.activation(
                        op_ident := ot[:], op[:], AF.Identity, bias=bias_sb[:, c, 2:3]
                    )
                    nc.sync.dma_start(out=out_d[ts(c, 128), ts(j, CHUNK)], in_=ot[:])

    nc.compile()
    return nc


_NC = None


def _get_nc():
    global _NC
    if _NC is None:
        _NC = _build()
    return _NC


def kernel(**inputs):
    x = np.asarray(inputs["x"], np.float32)
    source = np.asarray(inputs["source"], np.float32)
    Wq = np.asarray(inputs["Wq"], np.float32)
    bq = np.asarray(inputs["bq"], np.float32)
    Wk = np.asarray(inputs["Wk"], np.float32)
    bk = np.asarray(inputs["bk"], np.float32)
    Wv = np.asarray(inputs["Wv"], np.float32)
    bv = np.asarray(inputs["bv"], np.float32)
    Wm = np.asarray(inputs["Wm"], np.float64)
    W1 = np.asarray(inputs["W1"], np.float64)
    W2 = np.asarray(inputs["W2"], np.float32)
    b2 = np.asarray(inputs["b2"], np.float32)

    bf = ml_dtypes.bfloat16
    wqT = np.ascontiguousarray(Wq.reshape(H * DH, D).T).astype(bf).reshape(2, 128, D)
    wkT = np.ascontiguousarray(Wk.reshape(H * DH, D).T).astype(bf).reshape(2, 128, D)
    wvT = np.ascontiguousarray(Wv.reshape(H * DH, D).T).astype(bf).reshape(2, 128, D)
    # message-channel permutation (dh-major -> head-major) folded into Wm
    WmP = Wm.reshape(D, DH, H).transpose(0, 2, 1).reshape(D, D)
    # fold Wm into W1's message half; b1 and W1m@bm cancel in InstanceNorm
    W1mWm = W1[:, D:] @ WmP
    w1T = (
        np.vstack([W1[:, :D].T, W1mWm.T])
        .astype(np.float32)
        .astype(bf)
        .reshape(4, 128, 2 * D)
    )
    w2T = np.ascontiguousarray(W2.T).astype(bf).reshape(4, 128, D)
    bias = np.stack(
        [bq.reshape(D).astype(np.float32), bk.reshape(D).astype(np.float32),
         b2.reshape(D)], axis=1
    ).reshape(2, 128, 3)
    shared = {
        "wqT": wqT,
        "wkT": wkT,
        "wvT": wvT,
        "w1T": np.ascontiguousarray(w1T),
        "w2T": w2T,
        "bias": np.ascontiguousarray(bias),
        "bv": np.ascontiguousarray(bv.reshape(1, D)).astype(bf),
    }
    in_maps = []
    for b in range(B):
        m = dict(shared)
        m["x"] = np.ascontiguousarray(x[b]).astype(bf).reshape(2, 128, N)
        m["src"] = np.ascontiguousarray(source[b]).astype(bf).reshape(2, 128, M)
        in_maps.append(m)

    nc = _get_nc()
    res = run_bass_kernel_spmd(nc, in_maps, core_ids=list(range(B)))
    return np.stack([res.results[b]["out"] for b in range(B)], axis=0)


# revision 32
# speedup vs baseline: 1.0327x; 1.0327x over previous
"""AttentionalPropagation (SuperGlue-style GNN message passing) on 8 TRN2 NeuronCores.

Sharding: pure data parallel over the batch dim (B=8 -> one batch element per core).
Per-core computation (x, src are (256, 2048) slices; all matmuls in bf16, f32 accum):

  Q = WqS @ x + bq          (256, 2048)   stacked-head layout, c = h*64+dh
  K = WkS @ s + bk          (256, 2048)
  VT = s^T @ WvS^T + bv     (2048, 256)   keys on partitions (transposed layout)
  per head h: S^T[m,n] = K_h[:,m] . Q_h[:,n]  -> exp(S^T/8)  (no max-subtraction;
      scores are O(1) so exp is safe)
  msg_u[dh,n] = sum_m exp . VT[m, h*64+dh]  (col-packed head pairs)
  den[n] = sum_m exp                        (4-way col-packed ones-matmuls)
  msg = msg_u / den
  h1 = W1x @ x + (W1m@WmP) @ msg   (Wm folded into W1 on host; b1/bm-terms cancel
                                    in InstanceNorm)
  hn = relu(h1 - mean);  out = (W2 * rstd) @ hn + b2   (rstd>0 commutes with relu)

Scheduling: software-pipelined one n-chunk back AND interleaved at super-tile
granularity (scores for chunk j alternate with msg/den for chunk j-1 in the PE
stream), m-accumulation chains run reversed so Tile emits at most one semaphore
wait per chain.
"""

import os
import sys

for _p in ("/opt/trn_rl_repo",):
    if _p not in sys.path:
        sys.path.insert(0, _p)

import numpy as np
import ml_dtypes

import concourse.bass as bass
import concourse.mybir as mybir
from concourse import bacc
from concourse import library_config
from concourse.bass import ts
from concourse.tile import TileContext
from concourse.bass_utils import run_bass_kernel_spmd

F32 = mybir.dt.float32
BF16 = mybir.dt.bfloat16
AF = mybir.ActivationFunctionType
ALU = mybir.AluOpType

B, D, N, M, H, DH = 8, 256, 2048, 2048, 4, 64
EPS = 1e-5
NCH = 4  # n-chunks of 512
CHUNK = 512


def _build():
    nc = bacc.Bacc("TRN2", target_bir_lowering=False, debug=False, num_devices=8)

    x_d = nc.dram_tensor("x", [2, 128, N], BF16, kind="ExternalInput").ap()
    s_d = nc.dram_tensor("src", [2, 128, M], BF16, kind="ExternalInput").ap()
    wq_d = nc.dram_tensor("wqT", [2, 128, D], BF16, kind="ExternalInput").ap()
    wk_d = nc.dram_tensor("wkT", [2, 128, D], BF16, kind="ExternalInput").ap()
    wv_d = nc.dram_tensor("wvT", [2, 128, D], BF16, kind="ExternalInput").ap()
    w1_d = nc.dram_tensor("w1T", [4, 128, 2 * D], BF16, kind="ExternalInput").ap()
    w2_d = nc.dram_tensor("w2T", [4, 128, D], BF16, kind="ExternalInput").ap()
    # biases packed as columns: [bq, bk, b2]
    bias_d = nc.dram_tensor("bias", [2, 128, 3], F32, kind="ExternalInput").ap()
    bv_d = nc.dram_tensor("bv", [1, D], BF16, kind="ExternalInput").ap()
    out_d = nc.dram_tensor("out", [D, N], F32, kind="ExternalOutput").ap()

    with TileContext(nc) as tc:
        nc.gpsimd.load_library(library_config.attn)
        with (
            tc.tile_pool(name="const", bufs=1) as const,
            tc.tile_pool(name="data", bufs=1) as data,
            tc.tile_pool(name="reuse", bufs=2) as reuse,
            tc.tile_pool(name="exps", bufs=6) as exps,
            tc.tile_pool(name="small", bufs=2) as small,
            tc.tile_pool(name="msgn", bufs=4) as msgn,
            tc.tile_pool(name="ps_sc", bufs=2, space="PSUM") as ps_sc,
            tc.tile_pool(name="ps_shared", bufs=4, space="PSUM") as ps_shared,
        ):
            # ---- inputs + weights (few large DMAs; x/wq first for fast start) ----
            x_sb = data.tile([128, 2, N], BF16, name="x")
            wq_sb = const.tile([128, 2, D], BF16, name="wq")
            nc.sync.dma_start(out=x_sb[:], in_=x_d.rearrange("k p n -> p k n"))
            nc.sync.dma_start(out=wq_sb[:], in_=wq_d.rearrange("k p n -> p k n"))
            s_sb = reuse.tile([128, 2, M], BF16, name="s", tag="big")
            wk_sb = const.tile([128, 2, D], BF16, name="wk")
            wv_sb = const.tile([128, 2, D], BF16, name="wv")
            nc.sync.dma_start(out=s_sb[:], in_=s_d.rearrange("k p n -> p k n"))
            nc.sync.dma_start(out=wk_sb[:], in_=wk_d.rearrange("k p n -> p k n"))
            nc.sync.dma_start(out=wv_sb[:], in_=wv_d.rearrange("k p n -> p k n"))
            bias_sb = const.tile([128, 2, 3], F32, name="bias")
            nc.sync.dma_start(out=bias_sb[:], in_=bias_d.rearrange("k p n -> p k n"))
            bv_bc = const.tile([128, D], BF16, name="bvbc")
            bv_src = bass.AP(
                tensor=bv_d.tensor, offset=bv_d.offset, ap=[[0, 128]] + bv_d.ap[1:]
            )
            nc.sync.dma_start(out=bv_bc[:], in_=bv_src)
            w1_sb = const.tile([128, 4, 2 * D], BF16, name="w1")
            nc.sync.dma_start(out=w1_sb[:], in_=w1_d.rearrange("k p n -> p k n"))
            w2_sb = const.tile([128, 4, D], BF16, name="w2")
            nc.sync.dma_start(out=w2_sb[:], in_=w2_d.rearrange("k p n -> p k n"))
            eps_sb = const.tile([128, 1], F32, name="eps")
            nc.vector.memset(eps_sb[:], EPS)
            ones_sb = const.tile([128, 1], BF16, name="ones")
            nc.vector.memset(ones_sb[:], 1.0)

            # PE warmup: dummy matmuls on uninitialized SBUF while DMAs land,
            # so the HAM clock gate opens before the real QKV matmuls start
            dummy_sb = const.tile([128, 128], BF16, name="dummy")
            nc.vector.memset(dummy_sb[:], 0.0)
            wup = ps_shared.tile([128, 128], F32, name="wup", tag="sps")
            for _ in range(12):
                nc.tensor.matmul(wup[:], dummy_sb[:], dummy_sb[:],
                                 start=True, stop=True)

            # ---- QKV projections (weight-stationary: 1 LDW per 4 MMs) ----
            q_sb = data.tile([128, 2, N], BF16, name="q")
            k_sb = data.tile([128, 2, M], BF16, name="k")

            def emit_qk(c, skip_j0=False):
                for dst, w_sb, src_t, b_col in (
                    (q_sb, wq_sb, x_sb, 0),
                    (k_sb, wk_sb, s_sb, 1),
                ):
                    ps = [
                        ps_sc.tile([128, 2, CHUNK], F32, name="qk", tag="scps")
                        for _ in range(2)
                    ]
                    for k in range(2):
                        for j in range(NCH):
                            if skip_j0 and j == 0:
                                continue
                            nc.tensor.matmul(
                                ps[j // 2][:, j % 2, :],
                                w_sb[:, k, ts(c, 128)],
                                src_t[:, k, ts(j, CHUNK)],
                                start=(k == 0),
                                stop=(k == 1),
                            )
                    nc.vector.tensor_scalar_add(
                        dst[:, c, CHUNK if skip_j0 else 0 : 2 * CHUNK],
                        ps[0][:, 1:2, :] if skip_j0 else ps[0][:],
                        bias_sb[:, c, b_col : b_col + 1],
                    )
                    nc.vector.tensor_scalar_add(
                        dst[:, c, ts(1, 2 * CHUNK)],
                        ps[1][:],
                        bias_sb[:, c, b_col : b_col + 1],
                    )

            def emit_qk0_rest(c):
                emit_qk(c, skip_j0=True)

            # V^T: (m, c) layout, 65-wide per-head blocks with a ones column
            vT_sb = [data.tile([128, H, DH + 1], BF16, name=f"vT{t}")
                     for t in range(16)]

            def emit_vT(trange):
                for t in trange:
                    vp = ps_shared.tile([128, D], F32, name="vps", tag="sps")
                    for k in range(2):
                        nc.tensor.matmul(
                            vp[:],
                            s_sb[:, k, ts(t, 128)],
                            wv_sb[:, k, :],
                            start=(k == 0),
                            stop=(k == 1),
                        )
                    nc.vector.tensor_add(
                        vT_sb[t][:, :, 0:DH],
                        vp[:].rearrange("p (h d) -> p h d", h=H),
                        bv_bc[:].rearrange("p (h d) -> p h d", h=H),
                    )
                    nc.vector.memset(vT_sb[t][:, :, DH : DH + 1], 1.0)

            # ---- attention ----
            h1_sb = data.tile([128, 4, N], BF16, name="h1")
            stats_sb = data.tile([128, 4, NCH, 6], F32, name="stats")
            eS = {}  # (j, h, half) -> expS tile (128, 8, CHUNK)
            mn = {}  # (j, p) -> normalized msg pair tile (128, CHUNK)
            mps = {}  # (j, p) -> msg psum ; (j, 'd') -> den psum

            def emit_scores_super(j, p, s):
                # scores + exp for super-tile s (m-tiles 2s, 2s+1), head pair p
                if s == 0:
                    for h2 in range(2):
                        eS[(j, 2 * p + h2)] = exps.tile(
                            [128, 16, CHUNK], BF16, name="expS", tag="expS"
                        )
                scp = [
                    ps_sc.tile([128, 2, CHUNK], F32, name="sc", tag="scps")
                    for _ in range(2)
                ]
                for jj in range(2):
                    mt = 2 * s + jj
                    for h2 in range(2):
                        nc.tensor.matmul(
                            scp[h2][:, jj, :],
                            k_sb[ts(h2, DH), p, ts(mt, 128)],
                            q_sb[ts(h2, DH), p, ts(j, CHUNK)],
                            start=True,
                            stop=True,
                        )
                for h2 in range(2):
                    nc.scalar.activation(
                        eS[(j, 2 * p + h2)][:, 2 * s : 2 * s + 2, :],
                        scp[h2][:],
                        AF.Exp,
                        scale=1.0 / 8.0,
                    )

            def emit_msg_head(j, h):
                # augmented-V msg chain (psum row 64 = denominator)
                p, h2 = h // 2, h % 2
                if h2 == 0:
                    mn[(j, p)] = msgn.tile([128, CHUNK], BF16, name="mn", tag="mn")
                mp = ps_shared.tile([DH + 1, CHUNK], F32, name="msgps", tag="sps")
                for mt in range(16):
                    nc.tensor.matmul(
                        mp[:],
                        vT_sb[mt][:, h, :],
                        eS[(j, h)][:, mt, :],
                        start=(mt == 0),
                        stop=(mt == 15),
                    )
                del eS[(j, h)]
                den = small.tile([1, CHUNK], F32, name="den", tag="den")
                nc.vector.tensor_copy(den[:], mp[DH : DH + 1, :])
                rden = small.tile([1, CHUNK], F32, name="rden", tag="rden")
                nc.vector.reciprocal_approx_fast(rden[:], den[:])
                rbc = small.tile([DH, CHUNK], F32, name="rbc", tag="rbc")
                nc.gpsimd.partition_broadcast(rbc[:], rden[:])
                nc.vector.tensor_mul(mn[(j, p)][ts(h2, DH), :], mp[0:DH, :], rbc[:])

            def emit_norm_h1(j):
                # h1 = W1x @ x + W1mWm @ msg
                for o in range(4):
                    hp = ps_shared.tile([128, CHUNK], F32, name="h1ps", tag="sps")
                    for k in range(4):
                        rhs = (
                            x_sb[:, k, ts(j, CHUNK)] if k < 2 else mn[(j, k - 2)][:]
                        )
                        nc.tensor.matmul(
                            hp[:],
                            w1_sb[:, k, ts(o, 128)],
                            rhs,
                            start=(k == 0),
                            stop=(k == 3),
                        )
                    nc.vector.tensor_copy(h1_sb[:, o, ts(j, CHUNK)], hp[:])
                    nc.vector.bn_stats(
                        stats_sb[:, o, j, :], h1_sb[:, o, ts(j, CHUNK)]
                    )

            # ---- schedule ----
            for dst, w_sb, src_t, b_col in (
                (q_sb, wq_sb, x_sb, 0),
                (k_sb, wk_sb, s_sb, 1),
            ):
                pfx = ps_shared.tile([128, CHUNK], F32, name="pfx", tag="sps")
                for k in range(2):
                    nc.tensor.matmul(
                        pfx[:],
                        w_sb[:, k, 0:128],
                        src_t[:, k, 0:CHUNK],
                        start=(k == 0),
                        stop=(k == 1),
                    )
                nc.vector.tensor_scalar_add(
                    dst[:, 0, 0:CHUNK], pfx[:], bias_sb[:, 0, b_col : b_col + 1]
                )
            emit_scores_super(0, 0, 0)
            emit_qk0_rest(0)
            for s in range(1, 8):
                emit_scores_super(0, 0, s)
            emit_qk(1)
            for s in range(8):
                emit_scores_super(0, 1, s)
                emit_vT(range(2 * s, 2 * s + 2))
            for j in range(1, NCH - 1):
                for s in range(8):
                    emit_scores_super(j, 0, s)
                    emit_scores_super(j, 1, s)
                for h in range(4):
                    emit_msg_head(j - 1, h)
                emit_norm_h1(j - 1)
            jL = NCH - 1
            for s in range(8):
                emit_scores_super(jL, 0, s)
            for s in range(8):
                emit_scores_super(jL, 1, s)
            for h in range(4):
                emit_msg_head(jL - 1, h)
            emit_norm_h1(jL - 1)
            for h in range(4):
                emit_msg_head(jL, h)
            emit_norm_h1(jL)

            # ---- InstanceNorm (relu on DVE, rstd folded into W2) + W2 ----
            for _ in range(20):
                wup2 = ps_shared.tile([128, 128], F32, name="wup2", tag="sps")
                nc.tensor.matmul(wup2[:], dummy_sb[:], dummy_sb[:],
                                 start=True, stop=True)
            hn_sb = reuse.tile([128, 4, N], BF16, name="hn", tag="big")
            nmean = small.tile([128, 4], F32, name="nmean", tag="mean")
            var4 = small.tile([128, 4], F32, name="var4", tag="var4")
            for o in range(4):
                mv = small.tile([128, 2], F32, name="mv", tag="mv")
                nc.vector.bn_aggr(mv[:], stats_sb[:, o, :, :])
                nc.vector.tensor_scalar_mul(nmean[:, o : o + 1], mv[:, 0:1], -1.0)
                nc.vector.tensor_copy(var4[:, o : o + 1], mv[:, 1:2])
            lv4 = small.tile([128, 4], F32, name="lv4", tag="std4")
            nc.scalar.activation(lv4[:], var4[:], AF.Ln, bias=eps_sb[:])
            rstd4 = small.tile([128, 4], F32, name="rstd4", tag="rstd4")
            nc.scalar.activation(rstd4[:], lv4[:], AF.Exp, scale=-0.5)
            for o in range(4):
                nc.vector.tensor_scalar_mul(
                    w2_sb[:, o, :], w2_sb[:, o, :], rstd4[:, o : o + 1]
                )
            for j in range(NCH):
                for o in range(4):
                    if o % 2 == 0:
                        nc.scalar.activation(
                            hn_sb[:, o, ts(j, CHUNK)],
                            h1_sb[:, o, ts(j, CHUNK)],
                            AF.Relu,
                            bias=nmean[:, o : o + 1],
                        )
                    else:
                        nc.vector.tensor_scalar(
                            hn_sb[:, o, ts(j, CHUNK)],
                            h1_sb[:, o, ts(j, CHUNK)],
                            nmean[:, o : o + 1],
                            0.0,
                            op0=ALU.add,
                            op1=ALU.max,
                        )
                for c in range(2):
                    op = ps_shared.tile([128, CHUNK], F32, name="ops", tag="sps")
                    for ki, k in enumerate((3, 2, 1, 0)):
                        nc.tensor.matmul(
                            op[:],
                            w2_sb[:, k, ts(c, 128)],
                            hn_sb[:, k, ts(j, CHUNK)],
                            start=(ki == 0),
                            stop=(ki == 3),
                        )
                    ot = small.tile([128, CHUNK], F32, name="outt", tag="outt")
                    nc.scalar.activation(
                        op_ident := ot[:], op[:], AF.Identity, bias=bias_sb[:, c, 2:3]
                    )
                    nc.sync.dma_start(out=out_d[ts(c, 128), ts(j, CHUNK)], in_=ot[:])

    nc.compile()
    return nc


_NC = None


def _get_nc():
    global _NC
    if _NC is None:
        _NC = _build()
    return _NC


def kernel(**inputs):
    x = np.asarray(inputs["x"], np.float32)
    source = np.asarray(inputs["source"], np.float32)
    Wq = np.asarray(inputs["Wq"], np.float32)
    bq = np.asarray(inputs["bq"], np.float32)
    Wk = np.asarray(inputs["Wk"], np.float32)
    bk = np.asarray(inputs["bk"], np.float32)
    Wv = np.asarray(inputs["Wv"], np.float32)
    bv = np.asarray(inputs["bv"], np.float32)
    Wm = np.asarray(inputs["Wm"], np.float64)
    W1 = np.asarray(inputs["W1"], np.float64)
    W2 = np.asarray(inputs["W2"], np.float32)
    b2 = np.asarray(inputs["b2"], np.float32)

    bf = ml_dtypes.bfloat16
    wqT = np.ascontiguousarray(Wq.reshape(H * DH, D).T).astype(bf).reshape(2, 128, D)
    wkT = np.ascontiguousarray(Wk.reshape(H * DH, D).T).astype(bf).reshape(2, 128, D)
    wvT = np.ascontiguousarray(Wv.reshape(H * DH, D).T).astype(bf).reshape(2, 128, D)
    # message-channel permutation (dh-major -> head-major) folded into Wm
    WmP = Wm.reshape(D, DH, H).transpose(0, 2, 1).reshape(D, D)
    # fold Wm into W1's message half; b1 and W1m@bm cancel in InstanceNorm
    W1mWm = W1[:, D:] @ WmP
    w1T = (
        np.vstack([W1[:, :D].T, W1mWm.T])
        .astype(np.float32)
        .astype(bf)
        .reshape(4, 128, 2 * D)
    )
    w2T = np.ascontiguousarray(W2.T).astype(bf).reshape(4, 128, D)
    bias = np.stack(
        [bq.reshape(D).astype(np.float32), bk.reshape(D).astype(np.float32),
         b2.reshape(D)], axis=1
    ).reshape(2, 128, 3)
    shared = {
        "wqT": wqT,
        "wkT": wkT,
        "wvT": wvT,
        "w1T": np.ascontiguousarray(w1T),
        "w2T": w2T,
        "bias": np.ascontiguousarray(bias),
        "bv": np.ascontiguousarray(bv.reshape(1, D)).astype(bf),
    }
    in_maps = []
    for b in range(B):
        m = dict(shared)
        m["x"] = np.ascontiguousarray(x[b]).astype(bf).reshape(2, 128, N)
        m["src"] = np.ascontiguousarray(source[b]).astype(bf).reshape(2, 128, M)
        in_maps.append(m)

    nc = _get_nc()
    res = run_bass_kernel_spmd(nc, in_maps, core_ids=list(range(B)))
    return np.stack([res.results[b]["out"] for b in range(B)], axis=0)
